# revision 8
# baseline (speedup 1.0000x reference)
"""Trainium2 Bass kernel for DigitsCapsule dynamic routing — I-sharded.

Strategy (8 NeuronCores, model-parallel over input capsules I=1152 ->
144 per core, full batch B=512 on every core):
  Per routing iteration:
    s_part = x_loc @ (e_loc ⊙ w_loc)      (PE, full 128-deep contraction)
    s, S   = AllReduce([s_part; colsum(e_loc)])   (one ~115KB fp16 collective)
    v      = s · exp(0.5·ln q − ln(1+q) − 2·ln S),  q = ||s||²/S²
             (squash via Ln/Exp only — scalar engine stays on the
              natural_log_exp table set all kernel long: zero ~2.7µs
              ACT table reloads)
    T2     = x_locᵀ @ v                   (PE, full-B contraction)
    u_loc  = Σ_{k,O} w_loc ⊙ T2           (DVE on 144 rows only)
    b_loc += u_loc                        (LOCAL — no collective)
  Final iteration: same AllReduce; every core squashes the full batch and
  writes the full [B, O, L] output (client reads core 0) — no
  ReduceScatter, no 16-way permute DMAs.

The 144 local capsules split as 128 ("main") + 16 ("tail"). The tail is
packed (k, i16) into full 128-partition chunks; a constant 0/1 selection
matrix folds the partition-dim k-reduction back to [16, L] on the PE.
"""

import numpy as np

B, I, K, L, O = 512, 1152, 8, 16, 7
NC = 8
IL = I // NC          # 144 capsules per core
F = L * O             # 112
NB = B // 128         # 4 batch chunks
NS = 9                # contraction slots: 8 main (k, i128) + 1 tail (k, i16)
ITERS = 3
C2LNI = float(2.0 * np.log(I))   # 2·ln(1152): t=0 normalizer (S = I exactly)

_CACHE = {}


def _build(dt_key, repeat=1, abl=(), warm=10):
    import concourse.bacc as bacc
    import concourse.mybir as mybir
    import concourse.tile as tile

    DT = {"f32": mybir.dt.float32, "f16": mybir.dt.float16}[dt_key]
    F32 = mybir.dt.float32
    AF = mybir.ActivationFunctionType
    ALU = mybir.AluOpType
    AX = mybir.AxisListType

    nc = bacc.Bacc("TRN2", target_bir_lowering=False, debug=False, num_devices=NC)

    xT_d = nc.dram_tensor("xT", [NS * 128, B], DT, kind="ExternalInput")
    xG_d = nc.dram_tensor("xG", [B, NS * 128], DT, kind="ExternalInput")
    w0_d = nc.dram_tensor("w0", [128, K * F], DT, kind="ExternalInput")
    wt_d = nc.dram_tensor("wt", [128, F], DT, kind="ExternalInput")
    P_d = nc.dram_tensor("P", [128, 16], DT, kind="ExternalInput")
    Pt_d = nc.dram_tensor("Pt", [16, 128], DT, kind="ExternalInput")
    y_d = nc.dram_tensor("y", [128, NB * F], F32, kind="ExternalOutput")

    with tile.TileContext(nc) as tc:
        with (
            tc.tile_pool(name="const", bufs=1) as cpool,
            tc.tile_pool(name="work", bufs=2) as wpool,
            tc.tile_pool(name="small", bufs=2) as spool,
            tc.tile_pool(name="ps_s", bufs=2, space="PSUM") as ps_s,
            tc.tile_pool(name="ps_t2", bufs=1, space="PSUM") as ps_t2,
            tc.tile_pool(name="ps_sm", bufs=2, space="PSUM") as ps_sm,
            tc.tile_pool(name="ps_ln", bufs=1, space="PSUM") as ps_ln,
            tc.tile_pool(name="dram", bufs=2, space="DRAM") as dpool,
        ):
            # ---- one-time input loads, spread across DGE queues ----
            xT = cpool.tile([128, NS * B], DT, tag="xT")
            xt_src = xT_d[:].rearrange("(t p) b -> p t b", p=128)
            xt_dst = xT[:].rearrange("p (t b) -> p t b", t=NS)
            for h in range(3):
                lo, hi = h * 3, (h + 1) * 3
                nc.sync.dma_start(xt_dst[:, lo:hi], xt_src[:, lo:hi])

            xG = cpool.tile([128, NB * NS * 128], DT, tag="xG")
            xg_src = xG_d[:].rearrange("(c p) r -> p c r", p=128)
            xg_dst = xG[:].rearrange("p (c r) -> p c r", c=NB)
            for h in range(2):
                lo, hi = h * 2, (h + 1) * 2
                nc.gpsimd.dma_start(xg_dst[:, lo:hi], xg_src[:, lo:hi])

            w0 = cpool.tile([128, K * F], DT, tag="w0")
            nc.scalar.dma_start(w0[:], w0_d[:])
            wt = cpool.tile([128, F], DT, tag="wt")
            nc.scalar.dma_start(wt[:], wt_d[:])
            P = cpool.tile([128, 16], DT, tag="P")
            nc.scalar.dma_start(P[:], P_d[:])
            Pt = cpool.tile([16, 128], DT, tag="Pt")
            nc.scalar.dma_start(Pt[:], Pt_d[:])

            I32 = mybir.dt.int32
            ones = cpool.tile([128, 1], DT, tag="ones")
            nc.vector.memset(ones[:], 1.0)
            ones1 = cpool.tile([1, 128], F32, tag="ones1")
            nc.vector.memset(ones1[:], 1.0)
            magict = cpool.tile([128, 4 * L], I32, tag="magict")
            nc.vector.memset(magict[:], 0x5F3759DF)
            ones8th = cpool.tile([128, 1], DT, tag="ones8th")
            nc.vector.memset(ones8th[:], 0.125)
            srow = cpool.tile([1, F], DT, tag="srow")
            nc.vector.memset(srow[:], 0.0)

            b0 = cpool.tile([128, L], F32, tag="b0")
            bt = cpool.tile([16, L], F32, tag="bt")
            nc.vector.memset(b0[:], 0.0)
            nc.vector.memset(bt[:], 0.0)

            # warm the PE clock-gate during the input-DMA phase
            if "no_warm" not in abl:
                warm0 = cpool.tile([128, 64], DT, tag="warm0")
                nc.vector.memset(warm0[:], 0.0)
                wt0 = ps_sm.tile([1, 64], F32, tag="sm")
                for _ in range(14):
                    nc.tensor.matmul(wt0[:], ones[:], warm0[:],
                                     start=True, stop=True)

            for rep in range(repeat):
             for t in range(ITERS):
                # ---- coupling coefficients (unnormalized e = exp(b/B)) ----
                if t == 0 or "no_wc" in abl:
                    wc0s, wcts = w0, wt
                else:
                    e9 = wpool.tile([128, F], DT, tag="e9")
                    nc.scalar.activation(
                        e9[:].rearrange("p (l o) -> p l o", o=O),
                        b0[:].unsqueeze(2).to_broadcast((128, L, O)),
                        AF.Exp, scale=1.0 / B)
                    wc0 = wpool.tile([128, K * F], DT, tag="wc0")
                    nc.vector.tensor_tensor(
                        wc0[:].rearrange("p (k f) -> p k f", k=K),
                        w0[:].rearrange("p (k f) -> p k f", k=K),
                        e9[:].unsqueeze(1).to_broadcast((128, K, F)),
                        op=ALU.mult)
                    # tail: replicate bt across the 8 k-groups via a fp16
                    # selection matmul (Pt.T @ bt16), exp from PSUM
                    bt16 = spool.tile([16, L], DT, tag="bt16")
                    nc.vector.tensor_copy(bt16[:], bt[:])
                    btr_ps = ps_sm.tile([128, L], F32, tag="sm")
                    nc.tensor.matmul(btr_ps[:], Pt[:], bt16[:],
                                     start=True, stop=True)
                    etr = wpool.tile([128, F], DT, tag="etr")
                    nc.scalar.activation(
                        etr[:].rearrange("p (l o) -> p l o", o=O),
                        btr_ps[:].unsqueeze(2).to_broadcast((128, L, O)),
                        AF.Exp, scale=1.0 / B)
                    wct = wpool.tile([128, F], DT, tag="wct")
                    nc.vector.tensor_tensor(wct[:], wt[:], etr[:], op=ALU.mult)
                    # softmax normalizer from e9/etr colsums (etr is 8x
                    # k-replicated, so weight its ones by 1/8)
                    sm_ps = ps_sm.tile([1, F], F32, tag="sm")
                    nc.tensor.matmul(sm_ps[:], ones[:], e9[:],
                                     start=True, stop=False)
                    nc.tensor.matmul(sm_ps[:], ones8th[:], etr[:],
                                     start=False, stop=True)
                    nc.vector.tensor_copy(
                        srow[0:1, 0:L],
                        sm_ps[:].rearrange("p (l o) -> p o l", o=O)[:, 0, :])
                    wc0s, wcts = wc0, wct

                # ---- s_part = x_loc @ (e ⊙ w_loc) : [512, F] in 4 chunks ----
                s16 = wpool.tile([128, NB * F], DT, tag="s16")
                ar_in = dpool.tile([129, NB * F], DT, tag="ar_in")
                ar_out = dpool.tile([129, NB * F], DT, tag="ar_out")
                if "no_smm" in abl:
                    nc.vector.memset(s16[:], 0.001)
                    nc.sync.dma_start(ar_in[0:128, :], s16[:])
                else:
                    s_ps = ps_s.tile([128, NB * F], F32, tag="s_ps")
                    for bc in range(NB):
                        for s in range(NS):
                            rhs = (wc0s[:, (s * F):(s + 1) * F] if s < K
                                   else wcts[:])
                            nc.tensor.matmul(
                                s_ps[:, bc * F:(bc + 1) * F],
                                xT[:, s * B + bc * 128: s * B + (bc + 1) * 128],
                                rhs, start=(s == 0), stop=(s == NS - 1))
                        nc.vector.tensor_copy(
                            s16[:, bc * F:(bc + 1) * F],
                            s_ps[:, bc * F:(bc + 1) * F])
                    nc.sync.dma_start(ar_in[0:128, :], s16[:])
                if t > 0:
                    nc.gpsimd.dma_start(ar_in[128:129, 0:F], srow[:])

                # ---- collective: AllReduce every iteration ----
                nrow = 129 if t > 0 else 128
                if "no_ar" in abl:
                    nc.sync.dma_start(ar_out[0:nrow, :], ar_in[0:nrow, :])
                else:
                    nc.gpsimd.collective_compute(
                        "AllReduce", ALU.add,
                        replica_groups=[list(range(NC))],
                        ins=[ar_in[0:nrow, :].opt()],
                        outs=[ar_out[0:nrow, :].opt()])

                # keep PE's clock-gate open through the collective window:
                # dummy matmuls reading s16 (pins them to this window)
                if "no_warm" not in abl:
                    wrm = ps_sm.tile([1, NB * F], F32, tag="sm")
                    for _ in range(warm):
                        nc.tensor.matmul(wrm[:], ones[:], s16[:],
                                         start=True, stop=True)

                # ---- squash coefficients, all Ln/Exp (one ACT table set) --
                # g = exp(0.5·ln ssq − 2 ln S − ln(1+q)), v = s·g
                sfull = wpool.tile([128, NB * F], DT, tag="sfull")
                sq2 = wpool.tile([128, NB * F], F32, tag="sq2")
                ssq = spool.tile([128, 4 * L], F32, tag="ssq")
                nc.gpsimd.dma_start(sfull[:], ar_out[0:128, :])
                nc.vector.tensor_tensor(sq2[:], sfull[:], sfull[:],
                                        op=ALU.mult)
                nc.vector.tensor_reduce(
                    ssq[:], sq2[:].rearrange("p (q o) -> p q o", o=O),
                    axis=AX.X, op=ALU.add)
                # rsqrt(ssq) on DVE: bit-trick seed + 2 Newton steps — keeps
                # the scalar engine on one act-table set (no Sqrt/Ln loads)
                r = spool.tile([128, 4 * L], F32, tag="r")
                ri = r[:].bitcast(I32)
                nc.vector.tensor_scalar(ri, ssq[:].bitcast(I32), 1, None,
                                        op0=ALU.arith_shift_right)
                nc.vector.tensor_tensor(ri, magict[:], ri, op=ALU.subtract)
                hq = spool.tile([128, 4 * L], F32, tag="hq")
                nc.vector.tensor_scalar_mul(hq[:], ssq[:], 0.5)
                t1 = spool.tile([128, 4 * L], F32, tag="t1")
                for _ in range(2):
                    nc.vector.tensor_tensor(t1[:], r[:], r[:], op=ALU.mult)
                    nc.vector.tensor_tensor(t1[:], t1[:], hq[:], op=ALU.mult)
                    nc.vector.tensor_scalar(t1[:], t1[:], -1.0, 1.5,
                                            op0=ALU.mult, op1=ALU.add)
                    nc.vector.tensor_tensor(r[:], r[:], t1[:], op=ALU.mult)
                # g = sqrt(q)/((1+q)·S) with q = ssq/S²:
                #   g = (ssq·r) · rec(1+q) · invS²
                if t > 0:
                    svS = spool.tile([1, F], DT, tag="svS")
                    nc.gpsimd.dma_start(svS[:], ar_out[128:129, 0:F])
                    invS = spool.tile([1, L], F32, tag="invS")
                    nc.vector.reciprocal(invS[:], svS[0:1, 0:L])
                    iS64 = spool.tile([1, 4 * L], F32, tag="iS64")
                    nc.vector.tensor_tensor(
                        iS64[:].rearrange("p (c l) -> p c l", c=NB),
                        invS[:].unsqueeze(1).to_broadcast((1, NB, L)),
                        invS[:].unsqueeze(1).to_broadcast((1, NB, L)),
                        op=ALU.mult)
                    ib2 = ps_ln.tile([128, 4 * L], F32, tag="lnb")
                    nc.tensor.matmul(ib2[:], ones1[:], iS64[:],
                                     start=True, stop=True)
                q = spool.tile([128, 4 * L], F32, tag="q")
                if t == 0:
                    nc.vector.tensor_scalar_mul(q[:], ssq[:], 1.0 / (I * I))
                else:
                    nc.vector.tensor_tensor(q[:], ssq[:], ib2[:], op=ALU.mult)
                d1 = spool.tile([128, 4 * L], F32, tag="d1")
                nc.vector.tensor_scalar_add(d1[:], q[:], 1.0)
                rec = spool.tile([128, 4 * L], F32, tag="rec")
                nc.vector.reciprocal(rec[:], d1[:])
                g = spool.tile([128, 4 * L], F32, tag="g")
                nc.vector.tensor_tensor(g[:], ssq[:], r[:], op=ALU.mult)
                nc.vector.tensor_tensor(g[:], g[:], rec[:], op=ALU.mult)
                if t == 0:
                    nc.vector.tensor_scalar_mul(g[:], g[:], 1.0 / (I * I))
                else:
                    nc.vector.tensor_tensor(g[:], g[:], ib2[:], op=ALU.mult)

                if t == ITERS - 1:
                    # ---- final: full-batch v in (B, O, L) layout ----
                    v_out = wpool.tile([128, NB * F], F32, tag="v_out")
                    nc.vector.tensor_tensor(
                        v_out[:].rearrange("p (c o l) -> p c l o", o=O, l=L),
                        sfull[:].rearrange("p (c l o) -> p c l o", o=O, l=L),
                        g[:].rearrange("p (c l) -> p c l", l=L)
                        .unsqueeze(3).to_broadcast((128, NB, L, O)),
                        op=ALU.mult)
                    nc.scalar.dma_start(y_d[:], v_out[:])
                    continue

                v16 = wpool.tile([128, NB * F], DT, tag="v16")
                nc.vector.tensor_tensor(
                    v16[:].rearrange("p (q o) -> p q o", o=O),
                    sfull[:].rearrange("p (q o) -> p q o", o=O),
                    g[:].unsqueeze(2).to_broadcast((128, 4 * L, O)),
                    op=ALU.mult)

                if "no_u" in abl:
                    continue
                # ---- T2 = x_locᵀ @ v, agreement fused per k-half ----
                t2h = [ps_t2.tile([128, 4 * F], F32, tag=f"t2h{h}",
                                  name=f"t2h{h}")
                       for h in range(2)]
                t2t = ps_t2.tile([128, F], F32, tag="t2t")
                u0h = []
                for h in range(2):
                    for bc in range(NB):
                        xgb = xG[:, bc * NS * 128:]
                        vb = v16[:, bc * F:(bc + 1) * F]
                        for kk in range(4):
                            k = h * 4 + kk
                            nc.tensor.matmul(
                                t2h[h][:, kk * F:(kk + 1) * F],
                                xgb[:, k * 128:(k + 1) * 128], vb,
                                start=(bc == 0), stop=(bc == NB - 1))
                    prod = wpool.tile([128, 4 * F], DT, tag=f"prod{h}",
                                      name=f"prod{h}")
                    nc.vector.tensor_tensor(prod[:], t2h[h][:],
                                            w0[:, h * 4 * F:(h + 1) * 4 * F],
                                            op=ALU.mult)
                    uh = wpool.tile([128, L], F32, tag=f"u0{h}",
                                    name=f"u0{h}")
                    with nc.allow_low_precision("fp16 agreement; b in fp32"):
                        nc.vector.tensor_reduce(
                            uh[:],
                            prod[:].rearrange("p (k l o) -> p l k o",
                                              k=4, o=O),
                            axis=AX.XY, op=ALU.add)
                    u0h.append(uh)
                for bc in range(NB):
                    xgb = xG[:, bc * NS * 128:]
                    nc.tensor.matmul(
                        t2t[:], xgb[:, K * 128:NS * 128],
                        v16[:, bc * F:(bc + 1) * F],
                        start=(bc == 0), stop=(bc == NB - 1))
                prodt = wpool.tile([128, F], DT, tag="prodt")
                nc.vector.tensor_tensor(prodt[:], t2t[:], wt[:], op=ALU.mult)
                with nc.allow_low_precision("fp16 agreement; b re-acc fp32"):
                    qt = wpool.tile([128, L], DT, tag="qt")
                    nc.vector.tensor_reduce(
                        qt[:], prodt[:].rearrange("p (l o) -> p l o", o=O),
                        axis=AX.X, op=ALU.add)
                ut_ps = ps_sm.tile([16, L], F32, tag="sm")
                nc.tensor.matmul(ut_ps[:], P[:], qt[:], start=True, stop=True)
                if t == 0:
                    nc.vector.tensor_tensor(b0[:], u0h[0][:], u0h[1][:],
                                            op=ALU.add)
                    nc.vector.tensor_copy(bt[:], ut_ps[:])
                else:
                    nc.vector.tensor_add(b0[:], b0[:], u0h[0][:])
                    nc.vector.tensor_add(b0[:], b0[:], u0h[1][:])
                    nc.vector.tensor_add(bt[:], bt[:], ut_ps[:])

    nc.compile()
    return nc


def _get_nc(dt_key, repeat=1, abl=(), warm=10):
    key = (dt_key, repeat, tuple(sorted(abl)), warm)
    if key not in _CACHE:
        _CACHE[key] = _build(dt_key, repeat, abl, warm)
    return _CACHE[key]


def _prep_inputs(x, w, np_dt):
    """Per-core input maps for the I-sharded layout."""
    in_maps = []
    Pm = np.tile(np.eye(16, dtype=np_dt), (8, 1))          # [128, 16]
    Ptm = np.ascontiguousarray(Pm.T)                       # [16, 128]
    for c in range(NC):
        xl = x[:, c * IL:(c + 1) * IL, :].astype(np_dt)    # [512, 144, 8]
        main = xl[:, :128, :]                              # [512, 128, 8]
        tail = xl[:, 128:, :]                              # [512, 16, 8]
        # slots 0..7: (k, i128); slot 8: (k, i16)
        xT = np.empty((NS * 128, B), np_dt)
        for k in range(K):
            xT[k * 128:(k + 1) * 128, :] = main[:, :, k].T
        xT[K * 128:, :] = tail.transpose(2, 1, 0).reshape(128, B)
        xG = np.ascontiguousarray(xT.T)
        wl = w[c * IL:(c + 1) * IL].astype(np_dt)          # [144, 8, 16, 7]
        w0 = np.ascontiguousarray(wl[:128].reshape(128, K * F))
        wtl = np.ascontiguousarray(
            wl[128:].transpose(1, 0, 2, 3).reshape(128, F))
        in_maps.append({"xT": np.ascontiguousarray(xT), "xG": xG,
                        "w0": w0, "wt": wtl, "P": Pm, "Pt": Ptm})
    return in_maps


def kernel(x, w, _dt="f16", _trace=False):
    x = np.asarray(x, dtype=np.float32)
    w = np.asarray(w, dtype=np.float32)
    np_dt = {"f32": np.float32, "f16": np.float16}[_dt]

    nc = _get_nc(_dt)
    in_maps = _prep_inputs(x, w, np_dt)

    from concourse.bass_utils import run_bass_kernel_spmd
    res = run_bass_kernel_spmd(
        nc, in_maps, core_ids=list(range(NC)), trace=_trace)
    kernel.last_result = res
    # every core holds the full output; y[p, (c, o, l)] = v[c*128+p, o, l]
    y0 = res.results[0]["y"].reshape(128, NB, O, L)
    out = np.ascontiguousarray(y0.transpose(1, 0, 2, 3).reshape(B, O, L))
    return out.astype(np.float32)


kernel.last_result = None
